# revision 4
# baseline (speedup 1.0000x reference)
"""Causal self-attention (B=2, S=2048, D=1024, H=16) on 8 Trainium2 NeuronCores.

v3: v1's hardware-proven 2-heads-per-pass attention core (each QK head owns a
full PSUM bank — one tile_position per bank is a hardware constraint), with a
rebuilt schedule:

  - uniform blocks [(0,256),(256,256),(512,512),(1024,512),(1536,512)] x 2
    head-pair passes: 84 exp instructions vs v1's 110 (v1 split the last
    block 384+128 to shorten the drain; the staged tail below replaces that),
    saving ~5us of ScalarE instruction overhead (~185ns each).
  - priority-heap scheduling: QK matmuls and every PSUM-freeing evacuation
    run at priority 0; the TileScheduler then keeps the exp stream dense with
    instruction-granular preemption.
  - staged normalization: zps accumulator banks are evacuated to SBUF by a
    single high-priority copy so the next pass's PV can start immediately;
    reciprocal/broadcast/multiply run later off the critical loop.
  - x is staged in 8 chunks of 256 (contiguous DMAs); q/k/v projections and
    out-projection tiles are popped into per-tile slots by data deadline.
  - drain: the last pass's left-half norms (queries [1536:1920)) run after
    PV(14), overlapping tile-15's exp; out-proj j0-partials (head-pair 0)
    land early; final evacuations split across DVE and the idle ScalarE.
"""

import numpy as np

EMBED_DIM = 1024
NUM_HEADS = 16
HEAD_DIM = 64
BATCH = 2
N_CORES = 8
CORES_PER_BATCH = 4
HEADS_PER_CORE = 4
DQ = HEADS_PER_CORE * HEAD_DIM
VW = HEAD_DIM + 1
DV1 = HEADS_PER_CORE * VW  # 260
P = 128
WS = 16.0
EXP_SCALE = 1.0 / (WS * WS * 32.0)
SC = 512  # psum width for projections / outproj / lg
XC = 256  # x chunk width (DMA + projection granularity)
NXC = 2048 // XC
NT = 2048 // P

BLK = [(0, 256), (256, 256), (512, 512), (1024, 512), (1536, 512)]
NB = len(BLK)

N_WARM = 6

_NC_CACHE = {}


def _build_nc(seq):
    import concourse.bass as bass  # noqa: F401
    import concourse.mybir as mybir
    import concourse.tile as tile
    from concourse import bacc

    fp32 = mybir.dt.float32
    bf16 = mybir.dt.bfloat16
    fp8 = mybir.dt.float8e4
    AF = mybir.ActivationFunctionType
    ALU = mybir.AluOpType
    DR = mybir.MatmulPerfMode.DoubleRow

    S = seq
    ND = EMBED_DIM // P

    nc = bacc.Bacc("TRN2", target_bir_lowering=False, num_devices=N_CORES)

    xt8 = nc.declare_dram_parameter("xt8", [P, NXC, ND, XC], fp8, isOutput=False)
    xr8 = nc.declare_dram_parameter("xr8", [P, NXC, ND, XC], fp8, isOutput=False)
    wq8 = nc.declare_dram_parameter("wq8", [P, ND, DQ], fp8, isOutput=False)
    bq = nc.declare_dram_parameter("bq", [P, 2], fp32, isOutput=False)
    wk8 = nc.declare_dram_parameter("wk8", [P, ND, DQ], fp8, isOutput=False)
    bk = nc.declare_dram_parameter("bk", [P, 2], fp32, isOutput=False)
    wv8 = nc.declare_dram_parameter("wv8", [P, ND, DV1], fp8, isOutput=False)
    wvr8 = nc.declare_dram_parameter("wvr8", [P, ND, DV1], fp8, isOutput=False)
    bv1 = nc.declare_dram_parameter("bv1", [1, DV1], bf16, isOutput=False)
    wot = nc.declare_dram_parameter("wot", [P, 2, EMBED_DIM], bf16, isOutput=False)
    out = nc.declare_dram_parameter("out", [S, EMBED_DIM], bf16, isOutput=True)

    with tile.TileContext(nc) as tc:
        with (
            tc.tile_pool(name="const", bufs=1) as constp,
            tc.tile_pool(name="big", bufs=1) as big,
            tc.tile_pool(name="exp", bufs=24) as expp,
            tc.tile_pool(name="small", bufs=1) as small,
            tc.tile_pool(name="outsb", bufs=4) as outsb,
            tc.tile_pool(name="psA", bufs=2, space="PSUM") as psA,
            tc.tile_pool(name="psLG", bufs=2, space="PSUM") as psLG,
            tc.tile_pool(name="psZ", bufs=2, space="PSUM") as psZ,
        ):
            xt8_sb = big.tile([P, NXC, ND, XC], fp8, name="xt8_sb")
            xr8_sb = big.tile([P, NXC, ND, XC], fp8, name="xr8_sb")
            wq_sb = big.tile([P, ND, DQ], fp8, name="wq_sb")
            wk_sb = big.tile([P, ND, DQ], fp8, name="wk_sb")
            wv8_sb = big.tile([P, ND, DV1], fp8, name="wv8_sb")
            wvr8_sb = big.tile([P, ND, DV1], fp8, name="wvr8_sb")
            wot_sb = big.tile([P, DQ // P, EMBED_DIM], bf16, name="wot_sb")
            bq_sb = constp.tile([P, DQ // P], fp32, name="bq_sb")
            bk_sb = constp.tile([P, DQ // P], fp32, name="bk_sb")
            bv1_bf = constp.tile([1, DV1], bf16, name="bv1_bf")
            bv1_bc = constp.tile([P, DV1], bf16, name="bv1_bc")
            zeros8 = constp.tile([P, SC], fp8, name="zeros8")
            bzero = constp.tile([P, 1], fp32, name="bzero")
            qT8 = big.tile([P, 2, S], fp8, name="qT8")
            kT8 = big.tile([P, 2, S], fp8, name="kT8")
            v1 = big.tile([P, NT, DV1], bf16, name="v1")
            zT2 = big.tile([P, DQ // P, S], bf16, name="zT2")

            shuffle_id = list(range(32))
            zero_reg = nc.gpsimd.to_reg(0.0)

            # ---- projection units -----------------------------------------
            def unit_qk_proj(s0, which, j, evac="dve"):
                w = XC
                w_sb, b_sb, dstT = (
                    (wq_sb, bq_sb, qT8) if which == "q" else (wk_sb, bk_sb, kT8)
                )
                c = s0 // XC
                ps = psA.tile([P, SC], fp32, name="mmps")[:, :w]
                for dp in range(ND // 2):
                    nc.tensor.matmul(
                        ps[:],
                        w_sb[:, 2 * dp : 2 * dp + 2, j * P : (j + 1) * P],
                        xt8_sb[:, c, 2 * dp : 2 * dp + 2, :],
                        start=(dp == 0),
                        stop=(dp == ND // 2 - 1),
                        perf_mode=DR,
                    )
                if evac == "act":
                    nc.scalar.activation(
                        dstT[:, j, s0 : s0 + w],
                        ps[:],
                        AF.Identity,
                        bias=b_sb[:, j : j + 1],
                    )
                else:
                    # jump the DVE queue: this evac gates a block's QKs
                    with tc.high_priority():
                        nc.vector.affine_then_add(
                            dstT[:, j, s0 : s0 + w],
                            ps[:],
                            zeros8[:, :w],
                            scale=1.0,
                            bias=b_sb[:, j : j + 1],
                        )
                return ps

            def qk_units(c):  # q/k projection units for x chunk c
                return [lambda which=which, j=j: unit_qk_proj(c * XC, which, j)
                        for which in ("q", "k") for j in range(2)]

            def unit_v_proj(tt):
                c = tt // (XC // P)
                ps = psA.tile([P, SC], fp32, name="mmps")[:, :DV1]
                ttl = tt - c * (XC // P)
                groups = [(xt8_sb, wv8_sb), (xt8_sb, wvr8_sb), (xr8_sb, wv8_sb)]
                for gi, (xa, wa) in enumerate(groups):
                    for dp in range(ND // 2):
                        nc.tensor.matmul(
                            ps[:],
                            xa[:, c, 2 * dp : 2 * dp + 2, ttl * P : (ttl + 1) * P],
                            wa[:, 2 * dp : 2 * dp + 2, :],
                            start=(gi == 0 and dp == 0),
                            stop=(gi == 2 and dp == ND // 2 - 1),
                            perf_mode=DR,
                        )
                with tc.high_priority():
                    nc.vector.tensor_tensor(v1[:, tt, :], ps[:], bv1_bc[:], ALU.add)

            def outproj_units(i, depri=True, act_copy=False):
                """Two sub-units per 128-query tile (n=0,1) sharing one osb."""
                holder = {}

                def unit(n):
                    if n == 0:
                        holder["osb"] = outsb.tile([P, EMBED_DIM], bf16, name="osb")
                    ps = psA.tile([P, SC], fp32, name="mmps")
                    if depri:
                        with tc.high_priority(offset=-5000):
                            for j in range(DQ // P):
                                nc.tensor.matmul(
                                    ps[:],
                                    zT2[:, j, i * P : (i + 1) * P],
                                    wot_sb[:, j, n * SC : (n + 1) * SC],
                                    start=(j == 0),
                                    stop=(j == DQ // P - 1),
                                )
                    else:
                        for j in range(DQ // P):
                            nc.tensor.matmul(
                                ps[:],
                                zT2[:, j, i * P : (i + 1) * P],
                                wot_sb[:, j, n * SC : (n + 1) * SC],
                                start=(j == 0),
                                stop=(j == DQ // P - 1),
                            )
                    if act_copy and n == 0:
                        nc.scalar.activation(
                            holder["osb"][:, n * SC : (n + 1) * SC], ps[:],
                            AF.Identity, bias=bzero[:, 0:1],
                        )
                    else:
                        with tc.high_priority():
                            nc.vector.tensor_copy(
                                holder["osb"][:, n * SC : (n + 1) * SC], ps[:]
                            )
                    if n == 1:
                        nc.sync.dma_start(out[i * P : (i + 1) * P, :], holder["osb"])

                yield lambda: unit(0)
                yield lambda: unit(1)

            # ---- staged normalization ------------------------------------
            def stage_pair(ztiles, c0, c1, act=False):
                """Evacuate both zps accumulators into one SBUF tile at high
                priority so the PSUM banks free for the next pass's PV."""
                zs = small.tile([P, 2, SC], fp32, name="zstage", bufs=4)
                with tc.high_priority():
                    for i in range(2):
                        if act:
                            nc.scalar.activation(
                                zs[:VW, i, c0:c1], ztiles[i][:VW, c0:c1],
                                AF.Identity, bias=bzero[:VW, 0:1],
                            )
                        else:
                            nc.vector.tensor_copy(
                                zs[:VW, i, c0:c1], ztiles[i][:VW, c0:c1]
                            )
                return zs

            def recip_pair(zs, c0, c1):
                """One reciprocal + broadcast covering both heads' denoms."""
                ww = c1 - c0
                recip = small.tile([1, 2, SC], fp32, name="recip", bufs=6)
                rb = small.tile([HEAD_DIM, 2, SC], fp32, name="recip_bc",
                                bufs=6)
                nc.vector.reciprocal(recip[:, :, c0:c1], zs[VW - 1 : VW, :, c0:c1])
                nc.gpsimd.partition_broadcast(rb[:, :, c0:c1], recip[:, :, c0:c1])
                return rb

            def norm(h, s0, c0, c1, zs, rb):
                """Normalize head h over query cols [c0:c1) of its block."""
                ww = c1 - c0
                i = h % 2
                j = h // 2
                if h % 2 == 0:
                    nc.vector.tensor_tensor(
                        zT2[:HEAD_DIM, j, s0 + c0 : s0 + c1],
                        zs[:HEAD_DIM, i, c0:c1],
                        rb[:, i, c0:c1],
                        ALU.mult,
                    )
                else:
                    zodd = small.tile([HEAD_DIM, SC], bf16, name="zodd",
                                      bufs=8)[:, :ww]
                    nc.vector.tensor_tensor(
                        zodd[:], zs[:HEAD_DIM, i, c0:c1], rb[:, i, c0:c1],
                        ALU.mult
                    )
                    nc.vector.stream_shuffle(
                        zT2[HEAD_DIM:P, j, s0 + c0 : s0 + c1], zodd[:], shuffle_id
                    )

            # ---- attention pass: one head pair over one query block ------
            def attn_inline(b, hp, pops=None, prev_tail=None, tail_rate=4):
                s0, w = BLK[b]
                ntt = (s0 + w) // P
                diag0 = s0 // P
                zE = psZ.tile([P, SC], fp32, name="zps")
                zO = psZ.tile([P, SC], fp32, name="zps")
                ztiles = (zE, zO)
                exs = {}
                pops = pops or {}
                last = b == NB - 1 and hp == 1
                body_ntt = ntt - 1 if last else ntt
                stop_tt = ntt - 3 if last else ntt - 1

                def emit_pv(tt):
                    off = max(0, tt * P - s0)
                    ex = exs.pop(tt)
                    for i in range(2):
                        h = 2 * hp + i
                        nc.tensor.matmul(
                            ztiles[i][:VW, off:w],
                            v1[:, tt, h * VW : (h + 1) * VW],
                            ex[:, i, off:w],
                            start=(tt == 0),
                            stop=(tt == stop_tt),
                        )

                for tt in range(body_ntt):
                    off = max(0, tt * P - s0)
                    lg = psLG.tile([P, 2, SC], fp32, name="lgps")
                    ex = expp.tile([P, 2, SC], bf16, name="expt")
                    exs[tt] = ex
                    with tc.high_priority():
                        for i in range(2):
                            h = 2 * hp + i
                            nc.tensor.matmul(
                                lg[:, i, off:w],
                                kT8[32 * h : 32 * h + 32, :, tt * P : (tt + 1) * P],
                                qT8[32 * h : 32 * h + 32, :, s0 + off : s0 + w],
                                start=True,
                                stop=True,
                                perf_mode=DR,
                                tile_position=(32 * h, 0),
                            )
                    nc.scalar.activation(
                        ex[:, :, off:w], lg[:, :, off:w], AF.Exp, scale=EXP_SCALE
                    )
                    if tt >= diag0:
                        for i in range(2):
                            nc.gpsimd.affine_select(
                                out=ex[:, i, off : off + P],
                                in_=ex[:, i, off : off + P],
                                compare_op=ALU.is_gt,
                                fill=zero_reg,
                                base=1,
                                pattern=[[1, P]],
                                channel_multiplier=-1,
                            )
                    for f in pops.get(tt, ()):
                        f()
                    if tt >= 1 and prev_tail:
                        for _ in range(tail_rate):
                            if prev_tail:
                                prev_tail.pop(0)()
                    # lag-1 PV for non-diagonal tiles; diagonal tiles' PVs are
                    # deferred to the tail (their masks would stall the PE)
                    if tt >= 1 and tt - 1 < diag0:
                        emit_pv(tt - 1)
                while prev_tail:
                    prev_tail.pop(0)()

                if not last:
                    box = {}

                    def mk_stage():
                        box["zs"] = stage_pair(ztiles, 0, w)

                    def mk_recip():
                        box["rb"] = recip_pair(box["zs"], 0, w)

                    return (
                        [lambda tt=tt: emit_pv(tt)
                         for tt in range(diag0, ntt)]
                        + [mk_stage, mk_recip]
                        + [lambda i=i: norm(2 * hp + i, s0, 0, w, box["zs"],
                                            box["rb"])
                           for i in (1, 0)]
                    )

                # ---- last pass (block 4, hp=1): staggered drain -----------
                wl = w - P  # left split: query cols [0:wl), tile i 12..14
                # tile 15's QK/exp/mask first: the ACT stream finishes while
                # the left-half drain proceeds on PE/DVE/Pool
                tt = ntt - 1
                off15 = tt * P - s0
                lg = psLG.tile([P, 2, SC], fp32, name="lgps")
                ex15 = expp.tile([P, 2, SC], bf16, name="expt")
                for i in range(2):
                    h = 2 * hp + i
                    nc.tensor.matmul(
                        lg[:, i, off15:w],
                        kT8[32 * h : 32 * h + 32, :, tt * P : (tt + 1) * P],
                        qT8[32 * h : 32 * h + 32, :, s0 + off15 : s0 + w],
                        start=True,
                        stop=True,
                        perf_mode=DR,
                        tile_position=(32 * h, 0),
                    )
                nc.scalar.activation(
                    ex15[:, :, off15:w], lg[:, :, off15:w], AF.Exp,
                    scale=EXP_SCALE
                )
                for i in range(2):
                    nc.gpsimd.affine_select(
                        out=ex15[:, i, off15:w],
                        in_=ex15[:, i, off15:w],
                        compare_op=ALU.is_gt,
                        fill=zero_reg,
                        base=1,
                        pattern=[[1, P]],
                        channel_multiplier=-1,
                    )
                # PE warmers: keep the pstate up through the drain (run only
                # when the PE would otherwise idle)
                wlg = psLG.tile([P, 2, SC], fp32, name="lgps")
                with tc.high_priority(offset=-8000):
                    for _ in range(40):
                        nc.tensor.matmul(
                            wlg[:, 0, :P], zeros8[:, :P], zeros8[:, :P],
                            start=True, stop=True,
                        )
                # three-segment drain: zps groups closed at tile 13; tiles 14
                # and 15 accumulate in psA side banks, merged during staging.
                # Segment L = cols [0:256) -> oj(12,13), M = [256:384) ->
                # oj(14), R = [384:512) -> oj(15). Each segment's norms start
                # as soon as its last PV lands, overlapping the final exps.
                svB = psA.tile([P, SC], fp32, name="mmps")
                for i in range(2):
                    h = 2 * hp + i
                    nc.tensor.matmul(
                        svB[:VW, i * P : (i + 1) * P],
                        v1[:, tt, h * VW : (h + 1) * VW],
                        ex15[:, i, off15:w],
                        start=(i == 0),
                        stop=(i == 1),
                    )
                svA = psA.tile([P, SC], fp32, name="mmps")
                emit_pv(ntt - 4)
                emit_pv(ntt - 3)  # closes zps groups (stop_tt = 13)
                ex14 = exs.pop(ntt - 2)
                for i in range(2):
                    h = 2 * hp + i
                    nc.tensor.matmul(
                        svA[:VW, i * 2 * P : (i + 1) * 2 * P],
                        v1[:, ntt - 2, h * VW : (h + 1) * VW],
                        ex14[:, i, 2 * P : w],
                        start=(i == 0),
                        stop=(i == 1),
                    )
                # L segment
                zsL = stage_pair(ztiles, 0, 2 * P, act=True)
                rbL = recip_pair(zsL, 0, 2 * P)
                for i in (1, 0):
                    norm(2 * hp + i, s0, 0, 2 * P, zsL, rbL)
                for qi in (0, 1):
                    for u in outproj_units(ntt - 4 + qi, depri=False,
                                           act_copy=True):
                        u()
                # M segment: zps[2P:3P) + svA[:, i*2P : i*2P+P)
                zsM = small.tile([P, 2, SC], fp32, name="zstage", bufs=4)
                with tc.high_priority():
                    for i in range(2):
                        nc.scalar.activation(
                            zsM[:VW, i, 2 * P : 3 * P],
                            ztiles[i][:VW, 2 * P : 3 * P],
                            AF.Identity, bias=bzero[:VW, 0:1],
                        )
                        nc.vector.tensor_tensor(
                            zsM[:VW, i, 2 * P : 3 * P],
                            svA[:VW, i * 2 * P : i * 2 * P + P],
                            zsM[:VW, i, 2 * P : 3 * P],
                            ALU.add,
                        )
                rbM = recip_pair(zsM, 2 * P, 3 * P)
                for i in (1, 0):
                    norm(2 * hp + i, s0, 2 * P, 3 * P, zsM, rbM)
                for u in outproj_units(ntt - 2, depri=False, act_copy=True):
                    u()
                # R segment: zps[3P:4P) + svA[.., i*2P+P:..) + svB
                ps15 = [psA.tile([P, SC], fp32, name="mmps") for _ in range(2)]
                osb = outsb.tile([P, EMBED_DIM], bf16, name="osb")
                # out-proj j0 partial for tile 15 (head-pair 0 done long ago)
                for n in range(2):
                    nc.tensor.matmul(
                        ps15[n][:],
                        zT2[:, 0, tt * P : (tt + 1) * P],
                        wot_sb[:, 0, n * SC : (n + 1) * SC],
                        start=True,
                        stop=False,
                    )
                zsR = small.tile([P, 2, SC], fp32, name="zstage", bufs=4)
                with tc.high_priority():
                    for i in range(2):
                        nc.scalar.activation(
                            zsR[:VW, i, 3 * P : w], ztiles[i][:VW, 3 * P : w],
                            AF.Identity, bias=bzero[:VW, 0:1],
                        )
                        nc.vector.tensor_tensor(
                            zsR[:VW, i, 3 * P : w],
                            svA[:VW, i * 2 * P + P : (i + 1) * 2 * P],
                            zsR[:VW, i, 3 * P : w],
                            ALU.add,
                        )
                        nc.vector.tensor_tensor(
                            zsR[:VW, i, 3 * P : w],
                            svB[:VW, i * P : (i + 1) * P],
                            zsR[:VW, i, 3 * P : w],
                            ALU.add,
                        )
                rbR = recip_pair(zsR, 3 * P, w)
                for i in (1, 0):
                    norm(2 * hp + i, s0, 3 * P, w, zsR, rbR)
                for n in range(2):
                    nc.tensor.matmul(
                        ps15[n][:],
                        zT2[:, 1, tt * P : (tt + 1) * P],
                        wot_sb[:, 1, n * SC : (n + 1) * SC],
                        start=False,
                        stop=True,
                    )
                # evacuate: n=0 on DVE, n=1 on ACT (both idle now)
                nc.vector.tensor_copy(osb[:, :SC], ps15[0][:])
                nc.scalar.activation(osb[:, SC:], ps15[1][:], AF.Identity,
                                     bias=bzero[:, 0:1])
                nc.sync.dma_start(out[tt * P : (tt + 1) * P, :], osb[:])
                return []

            # ---- input DMAs: serial device, critical-first ---------------
            nc.sync.dma_start(wq_sb[:], wq8[:])
            nc.sync.dma_start(xt8_sb[:, 0], xt8[:, 0])
            nc.sync.dma_start(wk_sb[:], wk8[:])
            nc.scalar.dma_start(bq_sb[:], bq[:])
            nc.scalar.dma_start(bk_sb[:], bk[:])
            nc.sync.dma_start(bv1_bf[:], bv1[:])
            nc.sync.dma_start(xt8_sb[:, 1], xt8[:, 1])
            nc.gpsimd.memset(zeros8[:], 0.0)
            nc.gpsimd.memset(bzero[:], 0.0)
            nc.gpsimd.partition_broadcast(bv1_bc[:], bv1_bf[:])
            # PE warmup: ramp the pstate during the DMA lead-in
            wps = psLG.tile([P, 2, SC], fp32, name="lgps")
            for _ in range(N_WARM):
                nc.tensor.matmul(
                    wps[:, 0, :], zeros8[:, :P], zeros8[:], start=True, stop=True
                )
            nc.sync.dma_start(wv8_sb[:], wv8[:])
            nc.sync.dma_start(wvr8_sb[:], wvr8[:])
            nc.sync.dma_start(xr8_sb[:, 0], xr8[:, 0])
            for c in range(2, NXC):
                nc.sync.dma_start(xt8_sb[:, c], xt8[:, c])
                nc.sync.dma_start(xr8_sb[:, c - 1], xr8[:, c - 1])
            nc.sync.dma_start(xr8_sb[:, NXC - 1], xr8[:, NXC - 1])
            nc.sync.dma_start(wot_sb[:], wot[:])

            # ---- main schedule -------------------------------------------
            # lead-in: q/k for chunk 0 (block 0's queries + first key tiles);
            # k evacs go through the idle ScalarE
            for j in range(2):
                unit_qk_proj(0, "q", j, evac="dve")
            kps = []
            for j in range(2):
                ps = psZ.tile([P, SC], fp32, name="zps")[:, :XC]
                for dp in range(ND // 2):
                    nc.tensor.matmul(
                        ps[:],
                        wk_sb[:, 2 * dp : 2 * dp + 2, j * P : (j + 1) * P],
                        xt8_sb[:, 0, 2 * dp : 2 * dp + 2, :],
                        start=(dp == 0),
                        stop=(dp == ND // 2 - 1),
                        perf_mode=DR,
                    )
                kps.append(ps)
            for e0 in (0, P):
                for j in range(2):
                    nc.scalar.activation(
                        kT8[:, j, e0 : e0 + P],
                        kps[j][:, e0 : e0 + P],
                        AF.Identity,
                        bias=bk_sb[:, j : j + 1],
                    )
            for u in qk_units(1):
                u()

            vu = [lambda tt=tt: unit_v_proj(tt) for tt in range(NT)]
            ou = [u for i in range(12) for u in outproj_units(i)]

            qkc = {c: qk_units(c) for c in range(2, 8)}
            POPS = {
                0: {0: qkc[2][:2], 1: qkc[2][2:] + [vu[0]]},
                1: {0: [vu[1]] + qkc[3][:2], 1: qkc[3][2:]},
                2: {0: [vu[2]], 1: [vu[3]], 2: qkc[4][:2], 3: qkc[4][2:]},
                3: {0: [vu[4]], 1: [vu[5]], 2: qkc[5][:2] + [ou[0]],
                    3: qkc[5][2:] + [ou[1]]},
                4: {0: [vu[6]], 1: [vu[7]], 2: qkc[6][:2], 3: qkc[6][2:],
                    4: [ou[2]], 5: [ou[3]], 6: [ou[4]], 7: [ou[5]]},
                5: {0: [vu[8]], 1: [vu[9]], 2: qkc[7][:2], 3: qkc[7][2:],
                    4: [ou[6]], 5: [ou[7]]},
                6: {0: [vu[10]], 1: [vu[11]], 4: [ou[8]], 5: [ou[9]],
                    6: [ou[10]], 7: [ou[11]], 8: [ou[12]], 9: [ou[13]],
                    10: [ou[14]], 11: [ou[15]]},
                7: {0: [vu[12]], 1: [vu[13]]},
                8: {0: [vu[14]], 1: [vu[15]], 4: [ou[16]], 5: [ou[17]],
                    6: [ou[18]], 7: [ou[19]], 8: [ou[20]], 9: [ou[21]],
                    10: [ou[22]], 11: [ou[23]]},
                9: {},
            }

            t = None
            for p in range(2 * NB):
                b, hp = divmod(p, 2)
                t = attn_inline(b, hp, POPS.get(p), prev_tail=t)

    nc.finalize()
    return nc


def _get_nc(seq):
    if seq not in _NC_CACHE:
        _NC_CACHE[seq] = _build_nc(seq)
    return _NC_CACHE[seq]


def _perm_cols():
    perm = np.zeros(DQ, np.int64)
    for j in range(2):
        for p in range(P):
            h, r = divmod(p, 32)
            perm[j * P + p] = 64 * h + j * 32 + r
    return perm


def _pack(a, p=P):
    g = a.shape[0] // p
    return np.ascontiguousarray(a.reshape(g, p, -1).transpose(1, 0, 2).reshape(p, -1))


def shard_inputs(x, Wq, bq, Wk, bk, Wv, bv, Wo):
    import ml_dtypes

    bf = ml_dtypes.bfloat16
    f8 = ml_dtypes.float8_e4m3
    x = np.asarray(x, np.float32)
    S = x.shape[1]
    perm = _perm_cols()
    in_maps = []
    for c in range(N_CORES):
        b, g = divmod(c, CORES_PER_BATCH)
        sl = slice(g * DQ, (g + 1) * DQ)
        nd = EMBED_DIM // P
        xT = x[b].T.reshape(nd, P, S // XC, XC).transpose(1, 2, 0, 3)
        xT = np.ascontiguousarray(xT)
        xa = xT.astype(f8)  # A = fp8(x)
        xr = (xT - xa.astype(np.float32)).astype(f8)  # R = fp8(x - A)
        wq_l = _pack(Wq[:, sl][:, perm] * WS)
        bq_l = np.ascontiguousarray((bq[sl][perm] * WS).reshape(2, P).T)
        wk_l = _pack(Wk[:, sl][:, perm] * WS)
        bk_l = np.ascontiguousarray((bk[sl][perm] * WS).reshape(2, P).T)
        wv1 = np.zeros((EMBED_DIM, DV1), np.float32)
        bv1 = np.zeros((DV1,), np.float32)
        for h in range(HEADS_PER_CORE):
            col = g * DQ + h * HEAD_DIM
            wv1[:, h * VW : h * VW + HEAD_DIM] = Wv[:, col : col + HEAD_DIM]
            bv1[h * VW : h * VW + HEAD_DIM] = WS * bv[col : col + HEAD_DIM]
            bv1[h * VW + HEAD_DIM] = WS  # ones column -> denom row (x WS)
        wv_s = _pack(wv1 * WS)  # 16*Wv, packed
        wv8_a = wv_s.astype(f8)  # W8 = fp8(16*Wv)
        wvr_a = (wv_s - wv8_a.astype(np.float32)).astype(f8)  # WR
        in_maps.append(
            {
                "xt8": xa,
                "xr8": xr,
                "wq8": wq_l.astype(f8),
                "bq": bq_l,
                "wk8": wk_l.astype(f8),
                "bk": bk_l,
                "wv8": wv8_a,
                "wvr8": wvr_a,
                "bv1": bv1[None, :].astype(bf),
                "wot": _pack(np.ascontiguousarray(Wo[:, sl].T)).astype(bf),
            }
        )
    return in_maps


def kernel(x, Wq, bq, Wk, bk, Wv, bv, Wo):
    from concourse.bass_utils import run_bass_kernel_spmd

    x = np.asarray(x, np.float32)
    B, S, D = x.shape
    nc = _get_nc(S)
    in_maps = shard_inputs(x, Wq, bq, Wk, bk, Wv, bv, Wo)
    res = run_bass_kernel_spmd(nc, in_maps, core_ids=list(range(N_CORES)))
    out = np.zeros((B, S, D), np.float32)
    for c in range(N_CORES):
        b = c // CORES_PER_BATCH
        out[b] += res.results[c]["out"].astype(np.float32)
    return out
